# revision 24
# baseline (speedup 1.0000x reference)
"""Trainium2 Bass kernel for nn_MultiHeadModulator (8-core SPMD).

Math reformulation (exact): with a single query q = Wq@z_curr+bq,
  - dot scores:  score[l,h] = z[l]·A[:,h] + c[h],   A[:,h] = Wk[hb,:]^T @ q[hb]
  - rel scores fold into a per-(l,h) additive bias known on the host
  - value sum:   sum_l e[l,h]*v[l] = Wv @ (sum_l e[l,h]*z[l]) + (sum_l e[l,h])*bv
so the device only computes, per L-shard:
  score^T = A^T z^T   (PE, fp8 DoubleRow),  e^T = exp(scale*score + c_h) * fac
  U[h,:] += e^T z     (PE, fp8 DoubleRow),  S[h] via fused DVE reduce
and the host applies Wv/Wo and the softmax normalization to the tiny [8,512]
all-core sums.  Softmax runs without max-subtraction: scores are O(1) by
construction (validated |score| < 3).

Sharding: z_past split into 8 contiguous shards of 8192 rows, one per core.
The host ships each shard twice (feature-major for scores, row-major for U)
in fp8, pre-packed for DoubleRow access patterns.

Scheduling notes:
  - all bulk loads ride the sync HWDGE ring, dispatched before the compute
    loop in consumption order (zt0, zn0, zt1, ...); a_dr goes first on the
    scalar ring.  Putting bulk DMAs on the scalar ring stalls them behind
    blocked exp instructions; gpsimd/SWDGE starts ~10us late.
  - S is folded into the DVE correction-multiply via tensor_tensor_reduce
    (per-block partials, summed on host).
"""

import numpy as np
import ml_dtypes

import concourse.bass as bass  # noqa: F401  (engine namespaces live on the nc)
import concourse.mybir as mybir
import concourse.tile as tile
from concourse import bacc
from concourse.bass_utils import run_bass_kernel_spmd

HEADS = 8
REL_MAX = 64
DIM = 256
D2 = 512                      # flattened real feature dim
HD = DIM // HEADS             # 32 complex => 64 reals per head block
L_TOTAL = 65536
N_CORES = 8
L_SHARD = L_TOTAL // N_CORES  # 8192
N_BLOCKS = L_SHARD // 512     # 16 blocks of 512 rows
BLK_PER_SUPER = 2             # blocks per DMA (512 KB chunks)
N_SUPER = N_BLOCKS // BLK_PER_SUPER
SCALE = 1.0 / np.sqrt(HD)

FP8 = ml_dtypes.float8_e4m3   # == mybir.dt.float8e4 (trainium E4M3, max 240)
BF16 = ml_dtypes.bfloat16

TRACE = False                 # test.py can flip this for profiling runs
TRACE_KW = {}

_cached = {}


def _build_program(full_fac: bool):
    nc = bacc.Bacc(
        "TRN2", target_bir_lowering=False, debug=False, num_devices=N_CORES
    )
    DR = mybir.MatmulPerfMode.DoubleRow
    f8 = mybir.dt.float8e4

    ZT = nc.dram_tensor(
        "zt", [N_SUPER, 128, BLK_PER_SUPER, 2, 2, 512], f8, kind="ExternalInput"
    )
    ZN = nc.dram_tensor(
        "zn", [N_SUPER, 128, BLK_PER_SUPER, 2, 2, 512], f8, kind="ExternalInput"
    )
    AT = nc.dram_tensor("a_dr", [128, 2, 2, 16], f8, kind="ExternalInput")
    FAC = nc.dram_tensor(
        "fac", [8, L_SHARD if full_fac else 512], mybir.dt.bfloat16,
        kind="ExternalInput",
    )
    CB = nc.dram_tensor("cb", [8, 1], mybir.dt.float32, kind="ExternalInput")
    IDENT = nc.dram_tensor("ident", [8, 8], mybir.dt.bfloat16, kind="ExternalInput")
    ONES = nc.dram_tensor("ones", [128, 2, 1], f8, kind="ExternalInput")
    # single output: cols 0..511 = U, cols 512..527 = per-block S partials
    OUT = nc.dram_tensor("out", [8, 528], mybir.dt.float32, kind="ExternalOutput")

    with tile.TileContext(nc) as tc:
        with (
            tc.tile_pool(name="zt", bufs=N_SUPER) as zt_pool,
            tc.tile_pool(name="zn", bufs=N_SUPER) as zn_pool,
            tc.tile_pool(name="consts", bufs=1) as const_pool,
            tc.tile_pool(name="et", bufs=4) as et_pool,
            tc.tile_pool(name="e8", bufs=4) as e8_pool,
            tc.tile_pool(name="outs", bufs=1) as out_pool,
            tc.tile_pool(name="ps_sc", bufs=2, space="PSUM") as sc_pool,
            tc.tile_pool(name="ps_etp", bufs=2, space="PSUM") as etp_pool,
            tc.tile_pool(name="ps_acc", bufs=1, space="PSUM") as acc_pool,
        ):
            # everything rides the sync HWDGE ring in consumption order; the
            # tiny a_dr goes first so the ring's slow first-transfer
            # descriptor spin-up happens on 4KB, not on a 512KB chunk
            a_sb = const_pool.tile([128, 2, 2, 16], f8)
            nc.sync.dma_start(a_sb[:], AT[:])

            zt_tiles = [None] * N_SUPER
            zn_tiles = [None] * N_SUPER

            def load_super(sup):
                zt_s = zt_pool.tile([128, BLK_PER_SUPER, 2, 2, 512], f8)
                nc.sync.dma_start(zt_s[:], ZT[sup])
                zn_s = zn_pool.tile([128, BLK_PER_SUPER, 2, 2, 512], f8)
                nc.sync.dma_start(zn_s[:], ZN[sup])
                zt_tiles[sup] = zt_s
                zn_tiles[sup] = zn_s

            load_super(0)
            cb_sb = const_pool.tile([8, 1], mybir.dt.float32)
            nc.sync.dma_start(cb_sb[:], CB[:])
            id_sb = const_pool.tile([8, 8], mybir.dt.bfloat16)
            nc.sync.dma_start(id_sb[:], IDENT[:])
            ones_sb = const_pool.tile([128, 2, 1], f8)
            nc.sync.dma_start(ones_sb[:], ONES[:])
            fac_sb = const_pool.tile(
                [8, L_SHARD if full_fac else 512], mybir.dt.bfloat16
            )
            nc.sync.dma_start(fac_sb[:], FAC[:] if full_fac else FAC[:, 0:512])
            for sup in range(1, N_SUPER):
                load_super(sup)

            u_ps = acc_pool.tile([8, 512], mybir.dt.float32)
            s_ps = acc_pool.tile([8, 1], mybir.dt.float32)
            out_sb = out_pool.tile([8, 528], mybir.dt.float32)

            # one chain iteration (scores -> exp -> transpose -> cast -> U)
            # covers a whole 2-block super, halving cross-engine round trips
            srcs = {}
            e8s = {}

            def stage_a(sup):
                zt_s = zt_tiles[sup]
                sc2 = sc_pool.tile([8, BLK_PER_SUPER, 512], mybir.dt.float32)
                for j in range(BLK_PER_SUPER):
                    for cpair in range(2):
                        nc.tensor.matmul(
                            sc2[:, j],
                            a_sb[:, cpair, :, 0:8],
                            zt_s[:, j, cpair],
                            start=(cpair == 0),
                            stop=(cpair == 1),
                            perf_mode=DR,
                        )
                et2 = et_pool.tile([8, BLK_PER_SUPER, 512], mybir.dt.bfloat16, tag="et")
                if full_fac:
                    for j in range(BLK_PER_SUPER):
                        nc.scalar.activation(
                            et2[:, j],
                            sc2[:, j],
                            mybir.ActivationFunctionType.Exp,
                            bias=cb_sb[:, 0:1],
                            scale=float(SCALE),
                        )
                    etc2 = et_pool.tile(
                        [8, BLK_PER_SUPER, 512], mybir.dt.bfloat16, tag="etc"
                    )
                    for j in range(BLK_PER_SUPER):
                        b = BLK_PER_SUPER * sup + j
                        nc.vector.tensor_mul(
                            etc2[:, j], et2[:, j], fac_sb[:, 512 * b : 512 * (b + 1)]
                        )
                    srcs[sup] = etc2
                elif sup == 0:
                    # block 0 carries the rel-bias correction factors
                    nc.scalar.activation(
                        et2[:, 0],
                        sc2[:, 0],
                        mybir.ActivationFunctionType.Exp,
                        bias=cb_sb[:, 0:1],
                        scale=float(SCALE),
                    )
                    nc.scalar.activation(
                        et2[:, 1],
                        sc2[:, 1],
                        mybir.ActivationFunctionType.Exp,
                        bias=cb_sb[:, 0:1],
                        scale=float(SCALE),
                        accum_out=out_sb[:, 513:514],
                    )
                    etc0 = et_pool.tile([8, 512], mybir.dt.bfloat16, tag="etc")
                    nc.vector.tensor_mul(etc0[:], et2[:, 0], fac_sb[:, 0:512])
                    srcs[sup] = (etc0, et2)
                else:
                    nc.scalar.activation(
                        et2[:],
                        sc2[:],
                        mybir.ActivationFunctionType.Exp,
                        bias=cb_sb[:, 0:1],
                        scale=float(SCALE),
                        accum_out=out_sb[:, 513 + sup : 514 + sup],
                    )
                    srcs[sup] = et2

            def stage_b1(sup):
                src = srcs.pop(sup)
                etp = etp_pool.tile([128, 4 * BLK_PER_SUPER, 8], mybir.dt.bfloat16)
                for j in range(BLK_PER_SUPER):
                    if isinstance(src, tuple):
                        blk_src = src[0] if j == 0 else src[1][:, 1]
                    else:
                        blk_src = src[:, j]
                    for quad in range(4):
                        nc.tensor.transpose(
                            etp[:, 4 * j + quad],
                            blk_src[:, 128 * quad : 128 * (quad + 1)],
                            id_sb[:],
                        )
                e8 = e8_pool.tile([128, 4 * BLK_PER_SUPER, 16], f8)
                nc.vector.tensor_copy(e8[:, :, 0:8], etp[:])
                e8s[sup] = e8

            def stage_b2(sup):
                zn_s = zn_tiles[sup]
                e8 = e8s.pop(sup)
                for j in range(BLK_PER_SUPER):
                    b = BLK_PER_SUPER * sup + j
                    for s in range(2):
                        lhsT = e8[:, 4 * j + 2 * s : 4 * j + 2 * s + 2, 0:8]
                        nc.tensor.matmul(
                            u_ps[:],
                            lhsT,
                            zn_s[:, j, s],
                            start=(b == 0 and s == 0),
                            stop=(b == N_BLOCKS - 1 and s == 1),
                            perf_mode=DR,
                        )
                        if full_fac or b == 0:
                            last_s = (N_BLOCKS - 1 if full_fac else 0, 1)
                            nc.tensor.matmul(
                                s_ps[:],
                                lhsT,
                                ones_sb[:],
                                start=(b == 0 and s == 0),
                                stop=((b, s) == last_s),
                                perf_mode=DR,
                            )

            for step in range(N_SUPER + 2):
                if step < N_SUPER:
                    stage_a(step)
                if 1 <= step < N_SUPER + 1:
                    stage_b1(step - 1)
                if step >= 2:
                    stage_b2(step - 2)

            nc.vector.tensor_copy(out_sb[:, 0:512], u_ps[:])
            nc.vector.tensor_copy(out_sb[:, 512:513], s_ps[:])
            if full_fac:
                # accum_out unused: zero the remaining S columns
                nc.vector.memset(out_sb[:, 513:528], 0.0)
            else:
                nc.vector.memset(out_sb[:, 513 + N_SUPER : 528], 0.0)
            nc.sync.dma_start(OUT[:], out_sb[:])

    nc.compile()
    return nc


def _get_program(full_fac: bool):
    if full_fac not in _cached:
        _cached[full_fac] = _build_program(full_fac)
    return _cached[full_fac]


def kernel(curr_pos, z_curr, z_past, Wq, bq, Wk, bk, Wv, bv, Wo, bo, rel_bias):
    curr_pos = int(np.asarray(curr_pos))
    z_curr = np.asarray(z_curr, dtype=np.float32)
    z_past = np.asarray(z_past, dtype=np.float32)
    Wq = np.asarray(Wq, dtype=np.float32)
    bq = np.asarray(bq, dtype=np.float32)
    Wk = np.asarray(Wk, dtype=np.float32)
    bk = np.asarray(bk, dtype=np.float32)
    Wv = np.asarray(Wv, dtype=np.float32)
    bv = np.asarray(bv, dtype=np.float32)
    Wo = np.asarray(Wo, dtype=np.float32)
    bo = np.asarray(bo, dtype=np.float32)
    rel_bias = np.asarray(rel_bias, dtype=np.float32)

    # ---- host-side O(D^2) prep (f64) ----
    q = z_curr.reshape(-1).astype(np.float64) @ Wq.T.astype(np.float64) + bq
    A = np.zeros((D2, HEADS), np.float64)
    c = np.zeros(HEADS, np.float64)
    for h in range(HEADS):
        sl = slice(h * 2 * HD, (h + 1) * 2 * HD)
        A[:, h] = Wk[sl, :].T.astype(np.float64) @ q[sl]
        c[h] = bk[sl].astype(np.float64) @ q[sl]
    relflat = rel_bias.reshape(2 * REL_MAX + 1, D2).astype(np.float64)
    rb = np.stack(
        [
            relflat[:, h * 2 * HD : (h + 1) * 2 * HD] @ q[h * 2 * HD : (h + 1) * 2 * HD]
            for h in range(HEADS)
        ],
        axis=1,
    )  # [129, 8]
    idx = np.clip(
        curr_pos - L_TOTAL + np.arange(L_TOTAL) + REL_MAX, 0, 2 * REL_MAX
    ).astype(np.int64)

    z8 = np.clip(z_past.reshape(L_TOTAL, D2), -240.0, 240.0).astype(FP8)
    A8 = np.clip(A, -240.0, 240.0).astype(np.float32).astype(FP8)
    a_dr = np.zeros((128, 2, 2, 16), FP8)
    a_dr[:, :, :, 0:8] = A8.reshape(2, 2, 128, HEADS).transpose(2, 0, 1, 3)

    ident = np.eye(8, dtype=BF16)
    ones = np.ones((128, 2, 1), FP8)

    in_maps = []
    facs = []
    for core in range(N_CORES):
        zc = z8[core * L_SHARD : (core + 1) * L_SHARD]
        # zt[sup, p, blk, cpair, d, l] = zc[512*(2*sup+blk) + l, 256*cpair + 128d + p]
        zt = np.ascontiguousarray(
            zc.reshape(N_SUPER, BLK_PER_SUPER, 512, 2, 2, 128).transpose(
                0, 5, 1, 3, 4, 2
            )
        )
        # zn[sup, p, blk, s, d, f] = zc[512*(2*sup+blk) + 256s + 128d + p, f]
        zn = np.ascontiguousarray(
            zc.reshape(N_SUPER, BLK_PER_SUPER, 2, 2, 128, D2).transpose(
                0, 4, 1, 2, 3, 5
            )
        )
        idx_c = idx[core * L_SHARD : (core + 1) * L_SHARD]
        base = int(np.bincount(idx_c, minlength=2 * REL_MAX + 1).argmax())
        cb = ((c + rb[base]) * SCALE).astype(np.float32).reshape(HEADS, 1)
        fac = np.ascontiguousarray(
            np.exp((rb[idx_c] - rb[base]) * SCALE).T.astype(BF16)
        )
        facs.append(fac)
        in_maps.append(
            {
                "zt": zt,
                "zn": zn,
                "a_dr": a_dr,
                "fac": fac,
                "cb": cb,
                "ident": ident,
                "ones": ones,
            }
        )

    # fast path: correction factors are 1.0 outside block 0 on every core
    full_fac = any(
        not np.all(f[:, 512:] == np.asarray(1.0, BF16)) for f in facs
    )
    if not full_fac:
        for m in in_maps:
            m["fac"] = np.ascontiguousarray(m["fac"][:, 0:512])
    nc = _get_program(full_fac)
    res = run_bass_kernel_spmd(
        nc, in_maps, list(range(N_CORES)), trace=TRACE, **TRACE_KW
    )
    if TRACE:
        kernel.last_result = res

    U = np.zeros((HEADS, D2), np.float64)
    S = np.zeros(HEADS, np.float64)
    for r in res.results:
        o = np.asarray(r["out"], dtype=np.float64)
        U += o[:, 0:512]
        S += o[:, 512:528].sum(axis=1)

    hvec = np.zeros(D2, np.float64)
    for h in range(HEADS):
        sl = slice(h * 2 * HD, (h + 1) * 2 * HD)
        hvec[sl] = Wv[sl, :].astype(np.float64) @ (U[h] / S[h]) + bv[sl]
    out = hvec @ Wo.T.astype(np.float64) + bo
    return out.reshape(DIM, 2).astype(np.float32)


# revision 25
# speedup vs baseline: 1.2216x; 1.2216x over previous
"""Trainium2 Bass kernel for nn_MultiHeadModulator (8-core SPMD).

Math reformulation (exact): with a single query q = Wq@z_curr+bq,
  - dot scores:  score[l,h] = z[l]·A[:,h] + c[h],   A[:,h] = Wk[hb,:]^T @ q[hb]
  - rel scores fold into a per-(l,h) additive bias known on the host
  - value sum:   sum_l e[l,h]*v[l] = Wv @ (sum_l e[l,h]*z[l]) + (sum_l e[l,h])*bv
so the device only computes, per L-shard:
  score^T = A^T z^T   (PE, fp8 DoubleRow),  e^T = exp(scale*score + c_h) * fac
  U[h,:] += e^T z     (PE, fp8 DoubleRow),  S[h] via fused DVE reduce
and the host applies Wv/Wo and the softmax normalization to the tiny [8,512]
all-core sums.  Softmax runs without max-subtraction: scores are O(1) by
construction (validated |score| < 3).

Sharding: z_past split into 8 contiguous shards of 8192 rows, one per core.
The host ships each shard twice (feature-major for scores, row-major for U)
in fp8, pre-packed for DoubleRow access patterns.

Scheduling notes:
  - all bulk loads ride the sync HWDGE ring, dispatched before the compute
    loop in consumption order (zt0, zn0, zt1, ...); a_dr goes first on the
    scalar ring.  Putting bulk DMAs on the scalar ring stalls them behind
    blocked exp instructions; gpsimd/SWDGE starts ~10us late.
  - S is folded into the DVE correction-multiply via tensor_tensor_reduce
    (per-block partials, summed on host).
"""

import numpy as np
import ml_dtypes

import concourse.bass as bass  # noqa: F401  (engine namespaces live on the nc)
import concourse.mybir as mybir
import concourse.tile as tile
from concourse import bacc
from concourse.bass_utils import run_bass_kernel_spmd

HEADS = 8
REL_MAX = 64
DIM = 256
D2 = 512                      # flattened real feature dim
HD = DIM // HEADS             # 32 complex => 64 reals per head block
L_TOTAL = 65536
N_CORES = 8
L_SHARD = L_TOTAL // N_CORES  # 8192
N_BLOCKS = L_SHARD // 512     # 16 blocks of 512 rows
BLK_PER_SUPER = 4             # blocks per DMA (1 MB chunks)
N_SUPER = N_BLOCKS // BLK_PER_SUPER
SCALE = 1.0 / np.sqrt(HD)

FP8 = ml_dtypes.float8_e4m3   # == mybir.dt.float8e4 (trainium E4M3, max 240)
BF16 = ml_dtypes.bfloat16

TRACE = False                 # test.py can flip this for profiling runs
TRACE_KW = {}

_cached = {}


def _build_program(full_fac: bool):
    nc = bacc.Bacc(
        "TRN2", target_bir_lowering=False, debug=False, num_devices=N_CORES
    )
    DR = mybir.MatmulPerfMode.DoubleRow
    f8 = mybir.dt.float8e4

    ZT = nc.dram_tensor(
        "zt", [N_SUPER, 128, BLK_PER_SUPER, 2, 2, 512], f8, kind="ExternalInput"
    )
    ZN = nc.dram_tensor(
        "zn", [N_SUPER, 128, BLK_PER_SUPER, 2, 2, 512], f8, kind="ExternalInput"
    )
    AT = nc.dram_tensor("a_dr", [128, 2, 2, 16], f8, kind="ExternalInput")
    FAC = nc.dram_tensor(
        "fac", [8, L_SHARD if full_fac else 512], mybir.dt.bfloat16,
        kind="ExternalInput",
    )
    CB = nc.dram_tensor("cb", [8, 1], mybir.dt.float32, kind="ExternalInput")
    IDENT = nc.dram_tensor("ident", [8, 8], mybir.dt.bfloat16, kind="ExternalInput")
    ONES = nc.dram_tensor("ones", [128, 2, 1], f8, kind="ExternalInput")
    # single output: cols 0..511 = U, cols 512..527 = per-block S partials
    OUT = nc.dram_tensor("out", [8, 528], mybir.dt.float32, kind="ExternalOutput")

    with tile.TileContext(nc) as tc:
        with (
            tc.tile_pool(name="zt", bufs=N_SUPER) as zt_pool,
            tc.tile_pool(name="zn", bufs=N_SUPER) as zn_pool,
            tc.tile_pool(name="consts", bufs=1) as const_pool,
            tc.tile_pool(name="et", bufs=6) as et_pool,
            tc.tile_pool(name="e8", bufs=6) as e8_pool,
            tc.tile_pool(name="outs", bufs=1) as out_pool,
            tc.tile_pool(name="ps_sc", bufs=3, space="PSUM") as sc_pool,
            tc.tile_pool(name="ps_etp", bufs=3, space="PSUM") as etp_pool,
            tc.tile_pool(name="ps_acc", bufs=1, space="PSUM") as acc_pool,
        ):
            # a_dr rides the scalar ring (tiny, lands early); bulk zt on the
            # scalar ring's FIFO would stall behind exp instructions, so zt
            # leads the sync ring and zn follows it; small consts next.
            a_sb = const_pool.tile([128, 2, 2, 16], f8)
            nc.scalar.dma_start(a_sb[:], AT[:])

            zt_tiles = [None] * N_SUPER
            zn_tiles = [None] * N_SUPER

            def load_super(sup):
                zt_s = zt_pool.tile([128, BLK_PER_SUPER, 2, 2, 512], f8)
                nc.sync.dma_start(zt_s[:], ZT[sup])
                zn_s = zn_pool.tile([128, BLK_PER_SUPER, 2, 2, 512], f8)
                nc.sync.dma_start(zn_s[:], ZN[sup])
                zt_tiles[sup] = zt_s
                zn_tiles[sup] = zn_s

            load_super(0)
            cb_sb = const_pool.tile([8, 1], mybir.dt.float32)
            nc.sync.dma_start(cb_sb[:], CB[:])
            id_sb = const_pool.tile([8, 8], mybir.dt.bfloat16)
            nc.sync.dma_start(id_sb[:], IDENT[:])
            ones_sb = const_pool.tile([128, 2, 1], f8)
            nc.sync.dma_start(ones_sb[:], ONES[:])
            fac_sb = const_pool.tile(
                [8, L_SHARD if full_fac else 512], mybir.dt.bfloat16
            )
            nc.sync.dma_start(fac_sb[:], FAC[:])
            for sup in range(1, N_SUPER):
                load_super(sup)

            u_ps = acc_pool.tile([8, 512], mybir.dt.float32)
            s_ps = acc_pool.tile([8, 1], mybir.dt.float32)
            out_sb = out_pool.tile([8, 528], mybir.dt.float32)

            for b in range(N_BLOCKS):
                sup, blk = divmod(b, BLK_PER_SUPER)
                zt_t = zt_tiles[sup][:, blk]
                zn_t = zn_tiles[sup][:, blk]

                # score^T[h, l] for this block's 512 rows, K=512 via 2x DoubleRow
                sc = sc_pool.tile([8, 512], mybir.dt.float32)
                for cpair in range(2):
                    nc.tensor.matmul(
                        sc[:],
                        a_sb[:, cpair, :, 0:8],
                        zt_t[:, cpair],
                        start=(cpair == 0),
                        stop=(cpair == 1),
                        perf_mode=DR,
                    )

                et = et_pool.tile([8, 512], mybir.dt.bfloat16, tag="et")
                # for fac==1 blocks, S comes free from the exp's accum_out
                accum = (
                    {}
                    if (full_fac or b == 0)
                    else {"accum_out": out_sb[:, 512 + b : 513 + b]}
                )
                nc.scalar.activation(
                    et[:],
                    sc[:],
                    mybir.ActivationFunctionType.Exp,
                    bias=cb_sb[:, 0:1],
                    scale=float(SCALE),
                    **accum,
                )
                # rel-bias correction factors: only block 0 deviates from 1
                # in the common curr_pos regime (full_fac covers the rest)
                if full_fac or b == 0:
                    etc = et_pool.tile([8, 512], mybir.dt.bfloat16, tag="etc")
                    nc.vector.tensor_mul(
                        etc[:], et[:], fac_sb[:, 512 * b : 512 * (b + 1)]
                    )
                else:
                    etc = et

                # transpose e^T -> e[l,h] in 4x [8,128] chunks (PE via identity)
                etp = etp_pool.tile([128, 4, 8], mybir.dt.bfloat16)
                for quad in range(4):
                    nc.tensor.transpose(
                        etp[:, quad],
                        etc[:, 128 * quad : 128 * (quad + 1)],
                        id_sb[:],
                    )
                e8 = e8_pool.tile([128, 4, 16], f8)
                nc.vector.tensor_copy(e8[:, :, 0:8], etp[:])

                for s in range(2):
                    lhsT = e8[:, 2 * s : 2 * s + 2, 0:8]
                    nc.tensor.matmul(
                        u_ps[:],
                        lhsT,
                        zn_t[:, s],
                        start=(b == 0 and s == 0),
                        stop=(b == N_BLOCKS - 1 and s == 1),
                        perf_mode=DR,
                    )
                    if full_fac or b == 0:
                        last_s = (N_BLOCKS - 1 if full_fac else 0, 1)
                        nc.tensor.matmul(
                            s_ps[:],
                            lhsT,
                            ones_sb[:],
                            start=(b == 0 and s == 0),
                            stop=((b, s) == last_s),
                            perf_mode=DR,
                        )

            nc.vector.tensor_copy(out_sb[:, 0:512], u_ps[:])
            nc.vector.tensor_copy(out_sb[:, 512:513], s_ps[:])
            if full_fac:
                # accum_out unused: zero the remaining S columns
                nc.vector.memset(out_sb[:, 513:528], 0.0)
            nc.sync.dma_start(OUT[:], out_sb[:])

    nc.compile()
    return nc


def _get_program(full_fac: bool):
    if full_fac not in _cached:
        _cached[full_fac] = _build_program(full_fac)
    return _cached[full_fac]


def kernel(curr_pos, z_curr, z_past, Wq, bq, Wk, bk, Wv, bv, Wo, bo, rel_bias):
    curr_pos = int(np.asarray(curr_pos))
    z_curr = np.asarray(z_curr, dtype=np.float32)
    z_past = np.asarray(z_past, dtype=np.float32)
    Wq = np.asarray(Wq, dtype=np.float32)
    bq = np.asarray(bq, dtype=np.float32)
    Wk = np.asarray(Wk, dtype=np.float32)
    bk = np.asarray(bk, dtype=np.float32)
    Wv = np.asarray(Wv, dtype=np.float32)
    bv = np.asarray(bv, dtype=np.float32)
    Wo = np.asarray(Wo, dtype=np.float32)
    bo = np.asarray(bo, dtype=np.float32)
    rel_bias = np.asarray(rel_bias, dtype=np.float32)

    # ---- host-side O(D^2) prep (f64) ----
    q = z_curr.reshape(-1).astype(np.float64) @ Wq.T.astype(np.float64) + bq
    A = np.zeros((D2, HEADS), np.float64)
    c = np.zeros(HEADS, np.float64)
    for h in range(HEADS):
        sl = slice(h * 2 * HD, (h + 1) * 2 * HD)
        A[:, h] = Wk[sl, :].T.astype(np.float64) @ q[sl]
        c[h] = bk[sl].astype(np.float64) @ q[sl]
    relflat = rel_bias.reshape(2 * REL_MAX + 1, D2).astype(np.float64)
    rb = np.stack(
        [
            relflat[:, h * 2 * HD : (h + 1) * 2 * HD] @ q[h * 2 * HD : (h + 1) * 2 * HD]
            for h in range(HEADS)
        ],
        axis=1,
    )  # [129, 8]
    idx = np.clip(
        curr_pos - L_TOTAL + np.arange(L_TOTAL) + REL_MAX, 0, 2 * REL_MAX
    ).astype(np.int64)

    z8 = np.clip(z_past.reshape(L_TOTAL, D2), -240.0, 240.0).astype(FP8)
    A8 = np.clip(A, -240.0, 240.0).astype(np.float32).astype(FP8)
    a_dr = np.zeros((128, 2, 2, 16), FP8)
    a_dr[:, :, :, 0:8] = A8.reshape(2, 2, 128, HEADS).transpose(2, 0, 1, 3)

    ident = np.eye(8, dtype=BF16)
    ones = np.ones((128, 2, 1), FP8)

    in_maps = []
    facs = []
    for core in range(N_CORES):
        zc = z8[core * L_SHARD : (core + 1) * L_SHARD]
        # zt[sup, p, blk, cpair, d, l] = zc[512*(2*sup+blk) + l, 256*cpair + 128d + p]
        zt = np.ascontiguousarray(
            zc.reshape(N_SUPER, BLK_PER_SUPER, 512, 2, 2, 128).transpose(
                0, 5, 1, 3, 4, 2
            )
        )
        # zn[sup, p, blk, s, d, f] = zc[512*(2*sup+blk) + 256s + 128d + p, f]
        zn = np.ascontiguousarray(
            zc.reshape(N_SUPER, BLK_PER_SUPER, 2, 2, 128, D2).transpose(
                0, 4, 1, 2, 3, 5
            )
        )
        idx_c = idx[core * L_SHARD : (core + 1) * L_SHARD]
        base = int(np.bincount(idx_c, minlength=2 * REL_MAX + 1).argmax())
        cb = ((c + rb[base]) * SCALE).astype(np.float32).reshape(HEADS, 1)
        fac = np.ascontiguousarray(
            np.exp((rb[idx_c] - rb[base]) * SCALE).T.astype(BF16)
        )
        facs.append(fac)
        in_maps.append(
            {
                "zt": zt,
                "zn": zn,
                "a_dr": a_dr,
                "fac": fac,
                "cb": cb,
                "ident": ident,
                "ones": ones,
            }
        )

    # fast path: correction factors are 1.0 outside block 0 on every core
    full_fac = any(
        not np.all(f[:, 512:] == np.asarray(1.0, BF16)) for f in facs
    )
    if not full_fac:
        for m in in_maps:
            m["fac"] = np.ascontiguousarray(m["fac"][:, 0:512])
    nc = _get_program(full_fac)
    res = run_bass_kernel_spmd(
        nc, in_maps, list(range(N_CORES)), trace=TRACE, **TRACE_KW
    )
    if TRACE:
        kernel.last_result = res

    U = np.zeros((HEADS, D2), np.float64)
    S = np.zeros(HEADS, np.float64)
    for r in res.results:
        o = np.asarray(r["out"], dtype=np.float64)
        U += o[:, 0:512]
        S += o[:, 512:528].sum(axis=1)

    hvec = np.zeros(D2, np.float64)
    for h in range(HEADS):
        sl = slice(h * 2 * HD, (h + 1) * 2 * HD)
        hvec[sl] = Wv[sl, :].astype(np.float64) @ (U[h] / S[h]) + bv[sl]
    out = hvec @ Wo.T.astype(np.float64) + bo
    return out.reshape(DIM, 2).astype(np.float32)


# revision 26
# speedup vs baseline: 1.3032x; 1.0669x over previous
"""Trainium2 Bass kernel for nn_MultiHeadModulator (8-core SPMD).

Math reformulation (exact): with a single query q = Wq@z_curr+bq,
  - dot scores:  score[l,h] = z[l]·A[:,h] + c[h],   A[:,h] = Wk[hb,:]^T @ q[hb]
  - rel scores fold into a per-(l,h) additive bias known on the host
  - value sum:   sum_l e[l,h]*v[l] = Wv @ (sum_l e[l,h]*z[l]) + (sum_l e[l,h])*bv
so the device only computes, per L-shard:
  score^T = A^T z^T   (PE, fp8 DoubleRow),  e^T = exp(scale*score + c_h) * fac
  U[h,:] += e^T z     (PE, fp8 DoubleRow),  S[h] via fused DVE reduce
and the host applies Wv/Wo and the softmax normalization to the tiny [8,512]
all-core sums.  Softmax runs without max-subtraction: scores are O(1) by
construction (validated |score| < 3).

Sharding: z_past split into 8 contiguous shards of 8192 rows, one per core.
The host ships each shard twice (feature-major for scores, row-major for U)
in fp8, pre-packed for DoubleRow access patterns.

Scheduling notes:
  - all bulk loads ride the sync HWDGE ring, dispatched before the compute
    loop in consumption order (zt0, zn0, zt1, ...); a_dr goes first on the
    scalar ring.  Putting bulk DMAs on the scalar ring stalls them behind
    blocked exp instructions; gpsimd/SWDGE starts ~10us late.
  - S is folded into the DVE correction-multiply via tensor_tensor_reduce
    (per-block partials, summed on host).
"""

import numpy as np
import ml_dtypes

import concourse.bass as bass  # noqa: F401  (engine namespaces live on the nc)
import concourse.mybir as mybir
import concourse.tile as tile
from concourse import bacc
from concourse.bass_utils import run_bass_kernel_spmd

HEADS = 8
REL_MAX = 64
DIM = 256
D2 = 512                      # flattened real feature dim
HD = DIM // HEADS             # 32 complex => 64 reals per head block
L_TOTAL = 65536
N_CORES = 8
L_SHARD = L_TOTAL // N_CORES  # 8192
N_BLOCKS = L_SHARD // 512     # 16 blocks of 512 rows
BLK_PER_SUPER = 4             # blocks per DMA (1 MB chunks)
N_SUPER = N_BLOCKS // BLK_PER_SUPER
SCALE = 1.0 / np.sqrt(HD)

FP8 = ml_dtypes.float8_e4m3   # == mybir.dt.float8e4 (trainium E4M3, max 240)
BF16 = ml_dtypes.bfloat16

TRACE = False                 # test.py can flip this for profiling runs
TRACE_KW = {}

_cached = {}


def _build_program(full_fac: bool):
    nc = bacc.Bacc(
        "TRN2", target_bir_lowering=False, debug=False, num_devices=N_CORES
    )
    DR = mybir.MatmulPerfMode.DoubleRow
    f8 = mybir.dt.float8e4

    ZT = nc.dram_tensor(
        "zt", [N_SUPER, 128, BLK_PER_SUPER, 2, 2, 512], f8, kind="ExternalInput"
    )
    ZN = nc.dram_tensor(
        "zn", [N_SUPER, 128, BLK_PER_SUPER, 2, 2, 512], f8, kind="ExternalInput"
    )
    AT = nc.dram_tensor("a_dr", [128, 2, 2, 16], f8, kind="ExternalInput")
    FAC = nc.dram_tensor(
        "fac", [8, L_SHARD if full_fac else 512], mybir.dt.bfloat16,
        kind="ExternalInput",
    )
    CB = nc.dram_tensor("cb", [8, 1], mybir.dt.float32, kind="ExternalInput")
    IDENT = nc.dram_tensor("ident", [8, 8], mybir.dt.bfloat16, kind="ExternalInput")
    # single output: cols 0..511 = U, cols 512..527 = per-block S partials
    OUT = nc.dram_tensor("out", [8, 528], mybir.dt.float32, kind="ExternalOutput")

    with tile.TileContext(nc) as tc:
        with (
            tc.tile_pool(name="zt", bufs=N_SUPER) as zt_pool,
            tc.tile_pool(name="zn", bufs=N_SUPER) as zn_pool,
            tc.tile_pool(name="consts", bufs=1) as const_pool,
            tc.tile_pool(name="et", bufs=6) as et_pool,
            tc.tile_pool(name="e8", bufs=6) as e8_pool,
            tc.tile_pool(name="outs", bufs=1) as out_pool,
            tc.tile_pool(name="ps_sc", bufs=4, space="PSUM") as sc_pool,
            tc.tile_pool(name="ps_etp", bufs=3, space="PSUM") as etp_pool,
            tc.tile_pool(name="ps_acc", bufs=1, space="PSUM") as acc_pool,
        ):
            # a_dr rides the scalar ring (tiny, lands early); bulk zt on the
            # scalar ring's FIFO would stall behind exp instructions, so zt
            # leads the sync ring and zn follows it; small consts next.
            a_sb = const_pool.tile([128, 2, 2, 16], f8)
            nc.scalar.dma_start(a_sb[:], AT[:])

            zt_tiles = [None] * N_SUPER
            zn_tiles = [None] * N_SUPER

            def load_super(sup):
                zt_s = zt_pool.tile([128, BLK_PER_SUPER, 2, 2, 512], f8)
                nc.sync.dma_start(zt_s[:], ZT[sup])
                zn_s = zn_pool.tile([128, BLK_PER_SUPER, 2, 2, 512], f8)
                nc.sync.dma_start(zn_s[:], ZN[sup])
                zt_tiles[sup] = zt_s
                zn_tiles[sup] = zn_s

            load_super(0)
            cb_sb = const_pool.tile([8, 1], mybir.dt.float32)
            nc.sync.dma_start(cb_sb[:], CB[:])
            id_sb = const_pool.tile([8, 8], mybir.dt.bfloat16)
            nc.sync.dma_start(id_sb[:], IDENT[:])
            fac_sb = const_pool.tile(
                [8, L_SHARD if full_fac else 512], mybir.dt.bfloat16
            )
            nc.sync.dma_start(fac_sb[:], FAC[:])
            for sup in range(1, N_SUPER):
                load_super(sup)

            u_ps = acc_pool.tile([8, 512], mybir.dt.float32)
            out_sb = out_pool.tile([8, 528], mybir.dt.float32)

            for b in range(N_BLOCKS):
                sup, blk = divmod(b, BLK_PER_SUPER)
                zt_t = zt_tiles[sup][:, blk]
                zn_t = zn_tiles[sup][:, blk]

                # score^T[h, l] for this block's 512 rows, K=512 via 2x DoubleRow
                sc = sc_pool.tile([8, 512], mybir.dt.float32)
                for cpair in range(2):
                    nc.tensor.matmul(
                        sc[:],
                        a_sb[:, cpair, :, 0:8],
                        zt_t[:, cpair],
                        start=(cpair == 0),
                        stop=(cpair == 1),
                        perf_mode=DR,
                    )

                et = et_pool.tile([8, 512], mybir.dt.bfloat16, tag="et")
                # for fac==1 blocks, S comes free from the exp's accum_out
                accum = (
                    {}
                    if (full_fac or b == 0)
                    else {"accum_out": out_sb[:, 512 + b : 513 + b]}
                )
                nc.scalar.activation(
                    et[:],
                    sc[:],
                    mybir.ActivationFunctionType.Exp,
                    bias=cb_sb[:, 0:1],
                    scale=float(SCALE),
                    **accum,
                )
                # rel-bias correction factors: only block 0 deviates from 1
                # in the common curr_pos regime (full_fac covers the rest)
                if full_fac or b == 0:
                    etc = et_pool.tile([8, 512], mybir.dt.bfloat16, tag="etc")
                    nc.vector.tensor_mul(
                        etc[:], et[:], fac_sb[:, 512 * b : 512 * (b + 1)]
                    )
                    # S for corrected blocks: one DVE free-axis reduction
                    nc.vector.tensor_reduce(
                        out_sb[:, 512 + b : 513 + b],
                        etc[:],
                        axis=mybir.AxisListType.X,
                        op=mybir.AluOpType.add,
                    )
                else:
                    etc = et

                # transpose e^T -> e[l,h] in 4x [8,128] chunks (PE via identity)
                etp = etp_pool.tile([128, 4, 8], mybir.dt.bfloat16)
                for quad in range(4):
                    nc.tensor.transpose(
                        etp[:, quad],
                        etc[:, 128 * quad : 128 * (quad + 1)],
                        id_sb[:],
                    )
                e8 = e8_pool.tile([128, 4, 16], f8)
                nc.vector.tensor_copy(e8[:, :, 0:8], etp[:])

                for s in range(2):
                    nc.tensor.matmul(
                        u_ps[:],
                        e8[:, 2 * s : 2 * s + 2, 0:8],
                        zn_t[:, s],
                        start=(b == 0 and s == 0),
                        stop=(b == N_BLOCKS - 1 and s == 1),
                        perf_mode=DR,
                    )

            nc.vector.tensor_copy(out_sb[:, 0:512], u_ps[:])
            nc.sync.dma_start(OUT[:], out_sb[:])

    nc.compile()
    return nc


def _get_program(full_fac: bool):
    if full_fac not in _cached:
        _cached[full_fac] = _build_program(full_fac)
    return _cached[full_fac]


def kernel(curr_pos, z_curr, z_past, Wq, bq, Wk, bk, Wv, bv, Wo, bo, rel_bias):
    curr_pos = int(np.asarray(curr_pos))
    z_curr = np.asarray(z_curr, dtype=np.float32)
    z_past = np.asarray(z_past, dtype=np.float32)
    Wq = np.asarray(Wq, dtype=np.float32)
    bq = np.asarray(bq, dtype=np.float32)
    Wk = np.asarray(Wk, dtype=np.float32)
    bk = np.asarray(bk, dtype=np.float32)
    Wv = np.asarray(Wv, dtype=np.float32)
    bv = np.asarray(bv, dtype=np.float32)
    Wo = np.asarray(Wo, dtype=np.float32)
    bo = np.asarray(bo, dtype=np.float32)
    rel_bias = np.asarray(rel_bias, dtype=np.float32)

    # ---- host-side O(D^2) prep (f64) ----
    q = z_curr.reshape(-1).astype(np.float64) @ Wq.T.astype(np.float64) + bq
    A = np.zeros((D2, HEADS), np.float64)
    c = np.zeros(HEADS, np.float64)
    for h in range(HEADS):
        sl = slice(h * 2 * HD, (h + 1) * 2 * HD)
        A[:, h] = Wk[sl, :].T.astype(np.float64) @ q[sl]
        c[h] = bk[sl].astype(np.float64) @ q[sl]
    relflat = rel_bias.reshape(2 * REL_MAX + 1, D2).astype(np.float64)
    rb = np.stack(
        [
            relflat[:, h * 2 * HD : (h + 1) * 2 * HD] @ q[h * 2 * HD : (h + 1) * 2 * HD]
            for h in range(HEADS)
        ],
        axis=1,
    )  # [129, 8]
    idx = np.clip(
        curr_pos - L_TOTAL + np.arange(L_TOTAL) + REL_MAX, 0, 2 * REL_MAX
    ).astype(np.int64)

    z8 = np.clip(z_past.reshape(L_TOTAL, D2), -240.0, 240.0).astype(FP8)
    A8 = np.clip(A, -240.0, 240.0).astype(np.float32).astype(FP8)
    a_dr = np.zeros((128, 2, 2, 16), FP8)
    a_dr[:, :, :, 0:8] = A8.reshape(2, 2, 128, HEADS).transpose(2, 0, 1, 3)

    ident = np.eye(8, dtype=BF16)

    in_maps = []
    facs = []
    for core in range(N_CORES):
        zc = z8[core * L_SHARD : (core + 1) * L_SHARD]
        # zt[sup, p, blk, cpair, d, l] = zc[512*(2*sup+blk) + l, 256*cpair + 128d + p]
        zt = np.ascontiguousarray(
            zc.reshape(N_SUPER, BLK_PER_SUPER, 512, 2, 2, 128).transpose(
                0, 5, 1, 3, 4, 2
            )
        )
        # zn[sup, p, blk, s, d, f] = zc[512*(2*sup+blk) + 256s + 128d + p, f]
        zn = np.ascontiguousarray(
            zc.reshape(N_SUPER, BLK_PER_SUPER, 2, 2, 128, D2).transpose(
                0, 4, 1, 2, 3, 5
            )
        )
        idx_c = idx[core * L_SHARD : (core + 1) * L_SHARD]
        base = int(np.bincount(idx_c, minlength=2 * REL_MAX + 1).argmax())
        cb = ((c + rb[base]) * SCALE).astype(np.float32).reshape(HEADS, 1)
        fac = np.ascontiguousarray(
            np.exp((rb[idx_c] - rb[base]) * SCALE).T.astype(BF16)
        )
        facs.append(fac)
        in_maps.append(
            {
                "zt": zt,
                "zn": zn,
                "a_dr": a_dr,
                "fac": fac,
                "cb": cb,
                "ident": ident,
            }
        )

    # fast path: correction factors are 1.0 outside block 0 on every core
    full_fac = any(
        not np.all(f[:, 512:] == np.asarray(1.0, BF16)) for f in facs
    )
    if not full_fac:
        for m in in_maps:
            m["fac"] = np.ascontiguousarray(m["fac"][:, 0:512])
    nc = _get_program(full_fac)
    res = run_bass_kernel_spmd(
        nc, in_maps, list(range(N_CORES)), trace=TRACE, **TRACE_KW
    )
    if TRACE:
        kernel.last_result = res

    U = np.zeros((HEADS, D2), np.float64)
    S = np.zeros(HEADS, np.float64)
    for r in res.results:
        o = np.asarray(r["out"], dtype=np.float64)
        U += o[:, 0:512]
        S += o[:, 512:528].sum(axis=1)

    hvec = np.zeros(D2, np.float64)
    for h in range(HEADS):
        sl = slice(h * 2 * HD, (h + 1) * 2 * HD)
        hvec[sl] = Wv[sl, :].astype(np.float64) @ (U[h] / S[h]) + bv[sl]
    out = hvec @ Wo.T.astype(np.float64) + bo
    return out.reshape(DIM, 2).astype(np.float32)
